# revision 32
# baseline (speedup 1.0000x reference)
"""MoD-router FFN kernel for 8 TRN2 NeuronCores (self-contained). v9.

Math note: the reference applies softmax over a size-1 axis, which yields
all-ones scores for ANY input; jax.lax.top_k is stable, so the selected
token indices are always [0..NUM_TOKENS) per batch row. The router weights
(Wp, bp) therefore cannot affect the output, and the kernel computes

    out = gelu_tanh(x[:, :2048, :] @ W1 + b1) @ W2 + b2

Sharding: data-parallel over the 4*2048 = 8192 selected token rows ->
1024 rows per core. Two phases per core:
  A: H^T = gelu(W1^T @ X^T + b1), all 64 f-tiles resident in SBUF (bf16)
  B: out^T[d] = W2^T @ H^T via 64-matmul PSUM accumulation chains
     (d-outer), drained on the scalar engine with +b2 straight to SBUF
     and DMA'd out progressively.
Matmuls are bfloat16 except the first 256 contraction rows of FFN1 and the
first 768 of FFN2, which run as fp8e4(DoubleRow) matmuls; fp8 moving
operands are k-pair-interleaved so the PE reads both values per column at
once (rel err 1.813e-2 vs the 2e-2 gate; the numpy/ml_dtypes sim predicts
the HW error to ~5 decimals).
Scale bookkeeping: fp8 FFN1 operands carry x*16 and W1*64, so the bf16 W1
part is pre-scaled by 1024 on host and the gelu applies scale=1/1024; all
of W2 is pre-scaled by 128 (fp8 part stored as-is, bf16 part exact power
of 2) and the output drain applies scale=1/128.
"""

import numpy as np

B, S, D, F = 4, 4096, 2048, 8192
NUM_TOKENS = 2048
NCORES = 8
ROWS = (B * NUM_TOKENS) // NCORES     # 1024 rows per core
P = 128
KT8 = 2                               # k-subtiles over D handled in fp8
KT_B = D // P - KT8                   # 14 bf16 k-subtiles (D rows 256..2047)
FT = F // P                           # 64 f-tiles
WU = 4                                # warmup f-tiles (k-outer startup)
DT = D // P                           # 16 d-tiles
NCH = ROWS // 512                     # 2 row chunks of 512
FT8 = 6                               # f-subtiles of FFN2 handled in fp8
W2OFF = 4                             # first f-subtile held in the bf16 w2 param
FT_B = FT - W2OFF                     # 60 f-subtiles in the bf16 w2 param
                                      # (subtiles 4..5 overlap the fp8 part and
                                      # are skipped by the chain)
FSB = 4                               # w2 f-stream blocks per d (15 ft each)
SX, SW = 16.0, 64.0                   # fp8 pre-scales for x and W1
INV_S = 1.0 / (SX * SW)
S2 = 128.0                            # pre-scale for all of W2
INV_S2 = 1.0 / S2

_CACHE = {}


def _build():
    import concourse.bass as bass
    import concourse.mybir as mybir
    import concourse.tile as tile
    from concourse import bacc

    f32 = mybir.dt.float32
    bf16 = mybir.dt.bfloat16
    fp8 = mybir.dt.float8e4
    DR = mybir.MatmulPerfMode.DoubleRow

    nc = bacc.Bacc()
    xt = nc.declare_dram_parameter("xt", [KT_B, P, ROWS], bf16, isOutput=False)
    # fp8 moving operands are k-INTERLEAVED ([p, n, ks], pair adjacent in
    # memory) so the PE fetches both DoubleRow values in one 16-bit read
    xt8 = nc.declare_dram_parameter("xt8", [P, ROWS, KT8], fp8, isOutput=False)
    w1 = nc.declare_dram_parameter("w1", [FT, P, KT_B, P], bf16, isOutput=False)
    w18 = nc.declare_dram_parameter("w18", [FT, P, KT8, P], fp8, isOutput=False)
    # w2 grouped for phase B streaming: per (d, fsb) a [P, 15, P] bf16 block
    # (f-subtiles 4..63) plus a small fp8 block for f-subtiles 0..3
    w2 = nc.declare_dram_parameter("w2", [DT, FSB, P, FT_B // FSB, P], bf16,
                                   isOutput=False)
    w28 = nc.declare_dram_parameter("w28", [DT, P, FT8, P], fp8, isOutput=False)
    b1 = nc.declare_dram_parameter("b1", [P, FT], f32, isOutput=False)
    b2 = nc.declare_dram_parameter("b2", [P, DT], f32, isOutput=False)
    out = nc.declare_dram_parameter("out", [DT, P, ROWS], f32, isOutput=True)

    with tile.TileContext(nc) as tc:
        with (
            tc.tile_pool(name="resident", bufs=1) as res_pool,
            tc.tile_pool(name="wp", bufs=6) as wp,
            tc.tile_pool(name="w8p", bufs=6) as w8p,
            tc.tile_pool(name="w28p", bufs=4) as w28p,
            tc.tile_pool(name="stg", bufs=4) as stg,
            tc.tile_pool(name="ps1", bufs=4, space="PSUM") as ps1,
            tc.tile_pool(name="ps2", bufs=4, space="PSUM") as ps2,
        ):
            # residents: xt bf16 (28KB/part), xt8 fp8 (2KB/part), H (128KB/part)
            xt_all = res_pool.tile([P, KT_B, ROWS], bf16, name="xt_all")
            xt8_sb = res_pool.tile([P, ROWS, KT8], fp8, name="xt8sb")
            ht_sb = [res_pool.tile([P, ROWS], bf16, name=f"ht{ft}") for ft in range(FT)]
            # fp8 copies of ht f-subtiles 0..3, paired for DoubleRow
            ht8_sb = [res_pool.tile([P, ROWS, 2], fp8, name=f"ht8p{j}")
                      for j in range(FT8 // 2)]
            w1_warm = [wp.tile([P, KT_B * P], bf16, name=f"w1t_{ft}", tag="wp")
                       for ft in range(WU)]
            w18_warm = [w8p.tile([P, KT8, P], fp8, name=f"w18t_{ft}", tag="w8p")
                        for ft in range(WU)]
            b1_sb = res_pool.tile([P, FT], f32, name="b1sb")
            b2_sb = res_pool.tile([P, DT], f32, name="b2sb")

            # Startup DMAs: trigger instructions serialize at ~600ns each on
            # the Sync engine, so order them first-needed first (each chain
            # starts with its fp8 DoubleRow matmul: w18 + xt8 go first).
            nc.sync.dma_start(out=w18_warm[0][:], in_=w18[0])
            nc.sync.dma_start(out=w18_warm[1][:], in_=w18[1])
            nc.sync.dma_start(out=xt8_sb[:, 0:512, :], in_=xt8[:, 0:512, :])
            nc.sync.dma_start(out=xt8_sb[:, 512:1024, :], in_=xt8[:, 512:1024, :])
            nc.sync.dma_start(out=w1_warm[0][:], in_=w1[0].rearrange("p k c -> p (k c)"))
            nc.sync.dma_start(out=xt_all[:, 0, :], in_=xt[0])
            nc.sync.dma_start(out=w1_warm[1][:], in_=w1[1].rearrange("p k c -> p (k c)"))
            nc.sync.dma_start(out=xt_all[:, 1, :], in_=xt[1])
            nc.sync.dma_start(out=b1_sb[:], in_=b1[:])
            for k in range(2, KT_B):
                nc.sync.dma_start(out=xt_all[:, k, :], in_=xt[k])
            nc.sync.dma_start(out=w18_warm[2][:], in_=w18[2])
            nc.sync.dma_start(out=w18_warm[3][:], in_=w18[3])
            nc.sync.dma_start(out=w1_warm[2][:], in_=w1[2].rearrange("p k c -> p (k c)"))
            nc.sync.dma_start(out=w1_warm[3][:], in_=w1[3].rearrange("p k c -> p (k c)"))
            nc.sync.dma_start(out=b2_sb[:], in_=b2[:])

            def ffn1_chain(psum, w18_t, w1_t, n, for_k=None):
                """Issue one (ft, n) FFN1 chain; for_k limits to one bf16 k."""
                if for_k is None or for_k == -1:
                    nc.tensor.matmul(
                        psum[:], w18_t[:],
                        xt8_sb[:, n * 512:(n + 1) * 512, :].rearrange("p n k -> p k n"),
                        start=True, stop=False, perf_mode=DR,
                    )
                ks = range(KT_B) if for_k is None else (
                    [] if for_k == -1 else [for_k])
                for k in ks:
                    nc.tensor.matmul(
                        psum[:],
                        w1_t[:, k * P:(k + 1) * P],
                        xt_all[:, k, n * 512:(n + 1) * 512],
                        start=False, stop=(k == KT_B - 1),
                    )

            # ---------- phase A: H = gelu((x @ W1)/1024 + b1) ----------
            # warmup block: k-outer over 4 concurrent psum chains so matmuls
            # start as soon as xt_all[:, k] lands instead of waiting for all XT.
            for half in range(2):
                chains = [(half * 2 + i, n) for i in range(2) for n in range(NCH)]
                psums = {
                    c: ps1.tile([P, 512], f32, name=f"ps1w_{c[0]}_{c[1]}", tag="ps1")
                    for c in chains
                }
                for fs, n in chains:
                    ffn1_chain(psums[(fs, n)], w18_warm[fs], None, n, for_k=-1)
                for k in range(KT_B):
                    for fs, n in chains:
                        ffn1_chain(psums[(fs, n)], None, w1_warm[fs], n, for_k=k)
                for fs, n in chains:
                    # phase B consumes f-subtiles < FT8 in fp8 (DoubleRow) and
                    # the rest in bf16, so produce exactly the copy it reads
                    if fs < FT8:
                        nc.scalar.activation(
                            ht8_sb[fs // 2][:, n * 512:(n + 1) * 512, fs % 2],
                            psums[(fs, n)][:],
                            mybir.ActivationFunctionType.Gelu_apprx_tanh,
                            bias=b1_sb[:, fs:fs + 1], scale=INV_S,
                        )
                    else:
                        nc.scalar.activation(
                            ht_sb[fs][:, n * 512:(n + 1) * 512], psums[(fs, n)][:],
                            mybir.ActivationFunctionType.Gelu_apprx_tanh,
                            bias=b1_sb[:, fs:fs + 1], scale=INV_S,
                        )

            for ft in range(WU, FT):
                w18_t = w8p.tile([P, KT8, P], fp8, name=f"w18t_{ft}", tag="w8p")
                nc.sync.dma_start(out=w18_t[:], in_=w18[ft])
                w1_t = wp.tile([P, KT_B * P], bf16, name=f"w1t_{ft}", tag="wp")
                nc.sync.dma_start(out=w1_t[:], in_=w1[ft].rearrange("p k c -> p (k c)"))
                for n in range(NCH):
                    psum = ps1.tile([P, 512], f32, name=f"ps1_{ft}_{n}", tag="ps1")
                    ffn1_chain(psum, w18_t, w1_t, n)
                    if ft < FT8:
                        ht_out = ht8_sb[ft // 2][:, n * 512:(n + 1) * 512, ft % 2]
                    else:
                        ht_out = ht_sb[ft][:, n * 512:(n + 1) * 512]
                    nc.scalar.activation(
                        ht_out, psum[:],
                        mybir.ActivationFunctionType.Gelu_apprx_tanh,
                        bias=b1_sb[:, ft:ft + 1], scale=INV_S,
                    )

            # ---------- phase B: out[d] = (H @ W2*128)/128 + b2 ----------
            # per (d, n): 2 fp8 DoubleRow matmuls (f-subtiles 0..3) + 60 bf16
            for d in range(DT):
                w28_t = w28p.tile([P, FT8, P], fp8, name=f"w28t_{d}", tag="w28p")
                nc.sync.dma_start(out=w28_t[:], in_=w28[d])
                w2_sb = [wp.tile([P, (FT_B // FSB) * P], bf16, name=f"w2t_{d}_{fsb}",
                                 tag="wp") for fsb in range(FSB)]
                for fsb in range(FSB):
                    nc.sync.dma_start(
                        out=w2_sb[fsb][:],
                        in_=w2[d, fsb].rearrange("p k c -> p (k c)"))
                for n in range(NCH):
                    psum2 = ps2.tile([P, 512], f32, name=f"ps2_{d}_{n}", tag="ps2")
                    for j in range(FT8 // 2):
                        nc.tensor.matmul(
                            psum2[:],
                            w28_t[:, 2 * j:2 * j + 2, :],
                            ht8_sb[j][:, n * 512:(n + 1) * 512, :].rearrange("p n k -> p k n"),
                            start=(j == 0), stop=False, perf_mode=DR,
                        )
                    for fs in range(FT8, FT):
                        fsb, fi = divmod(fs - W2OFF, FT_B // FSB)
                        nc.tensor.matmul(
                            psum2[:],
                            w2_sb[fsb][:, fi * P:(fi + 1) * P],
                            ht_sb[fs][:, n * 512:(n + 1) * 512],
                            start=False, stop=(fs == FT - 1),
                        )
                    if d == DT - 1 and n == NCH - 1:
                        # last drain: halves pipeline ACTIVATE with DMA
                        for h in range(2):
                            o_h = stg.tile([P, 256], f32, name=f"oh_{h}", tag="stgh")
                            nc.scalar.activation(
                                o_h[:], psum2[:, h * 256:(h + 1) * 256],
                                mybir.ActivationFunctionType.Identity,
                                bias=b2_sb[:, d:d + 1], scale=INV_S2,
                            )
                            nc.sync.dma_start(
                                out=out[d, :, n * 512 + h * 256:n * 512 + (h + 1) * 256],
                                in_=o_h[:])
                    else:
                        o_sb = stg.tile([P, 512], f32, name=f"o_{d}_{n}", tag="stg")
                        nc.scalar.activation(
                            o_sb[:], psum2[:],
                            mybir.ActivationFunctionType.Identity,
                            bias=b2_sb[:, d:d + 1], scale=INV_S2,
                        )
                        nc.sync.dma_start(out=out[d, :, n * 512:(n + 1) * 512], in_=o_sb[:])

    nc.compile()
    return nc


def _get_nc():
    if "nc" not in _CACHE:
        _CACHE["nc"] = _build()
    return _CACHE["nc"]


def _prep_in_maps(x, W1, b1, W2, b2):
    """Host-side shard + layout prep. Returns in_maps for the 8 cores."""
    import ml_dtypes

    BF16 = ml_dtypes.bfloat16
    FP8 = ml_dtypes.float8_e4m3
    x = np.asarray(x, dtype=np.float32)
    W1 = np.asarray(W1, dtype=np.float32)
    W2 = np.asarray(W2, dtype=np.float32)
    b1 = np.asarray(b1, dtype=np.float32)
    b2 = np.asarray(b2, dtype=np.float32)

    KCUT = KT8 * P                                               # 256
    xs = x[:, :NUM_TOKENS, :].reshape(B * NUM_TOKENS, D)         # [8192, 2048]
    # bf16 W1 part pre-scaled by SX*SW (power of 2: exact in bf16)
    w1h = np.ascontiguousarray(
        (W1[KCUT:] * (SX * SW)).reshape(KT_B, P, FT, P)
        .transpose(2, 1, 0, 3)).astype(BF16)                     # [ft, p, k, c]
    w18h = np.ascontiguousarray(
        (W1[:KCUT] * SW).reshape(KT8, P, FT, P)
        .transpose(2, 1, 0, 3)).astype(FP8)                      # [ft, p, k8, c]
    # all of W2 carries the S2 scale (exact power of 2 for the bf16 part);
    # the bf16 param keeps f-subtiles 4..63 (4..5 unused), fp8 covers 0..5
    w2h = np.ascontiguousarray(
        (W2[W2OFF * P:] * S2).reshape(FSB, FT_B // FSB, P, DT, P)
        .transpose(3, 0, 2, 1, 4)).astype(BF16)                  # [d, blk, p, fi, c]
    w28h = np.ascontiguousarray(
        (W2[:FT8 * P] * S2).reshape(FT8, P, DT, P)
        .transpose(2, 1, 0, 3)).astype(FP8)                      # [d, p, ks, c]
    b1h = np.ascontiguousarray(b1.reshape(FT, P).T)              # [p, ft]
    b2h = np.ascontiguousarray(b2.reshape(DT, P).T)              # [p, d]

    in_maps = []
    for c in range(NCORES):
        xc = xs[c * ROWS:(c + 1) * ROWS]                         # [1024, 2048]
        xth = np.ascontiguousarray(
            xc[:, KCUT:].T.reshape(KT_B, P, ROWS)).astype(BF16)
        # [p, n, k8]: the two DoubleRow values adjacent in memory per column
        xt8h = np.ascontiguousarray(
            (xc[:, :KCUT] * SX).reshape(ROWS, KT8, P)
            .transpose(2, 0, 1)).astype(FP8)
        in_maps.append({"xt": xth, "xt8": xt8h, "w1": w1h, "w18": w18h,
                        "w2": w2h, "w28": w28h, "b1": b1h, "b2": b2h})
    return in_maps


def _gather_out(results):
    out = np.empty((B * NUM_TOKENS, D), dtype=np.float32)
    for c in range(NCORES):
        oc = results[c]["out"]                                   # [d, p, n]
        out[c * ROWS:(c + 1) * ROWS] = oc.reshape(D, ROWS).T
    return out.reshape(B, NUM_TOKENS, D)


def kernel(x, Wp, bp, W1, b1, W2, b2, **_unused):
    from concourse.bass_utils import run_bass_kernel_spmd

    in_maps = _prep_in_maps(x, W1, b1, W2, b2)
    nc = _get_nc()
    res = run_bass_kernel_spmd(nc, in_maps, list(range(NCORES)))
    return _gather_out(res.results)


# revision 34
# speedup vs baseline: 1.0020x; 1.0020x over previous
"""MoD-router FFN kernel for 8 TRN2 NeuronCores (self-contained). v9.

Math note: the reference applies softmax over a size-1 axis, which yields
all-ones scores for ANY input; jax.lax.top_k is stable, so the selected
token indices are always [0..NUM_TOKENS) per batch row. The router weights
(Wp, bp) therefore cannot affect the output, and the kernel computes

    out = gelu_tanh(x[:, :2048, :] @ W1 + b1) @ W2 + b2

Sharding: data-parallel over the 4*2048 = 8192 selected token rows ->
1024 rows per core. Two phases per core:
  A: H^T = gelu(W1^T @ X^T + b1), all 64 f-tiles resident in SBUF (bf16)
  B: out^T[d] = W2^T @ H^T via 64-matmul PSUM accumulation chains
     (d-outer), drained on the scalar engine with +b2 straight to SBUF
     and DMA'd out progressively.
Matmuls are bfloat16 except the first 256 contraction rows of FFN1 and the
first 768 of FFN2, which run as fp8e4(DoubleRow) matmuls; fp8 moving
operands are k-pair-interleaved so the PE reads both values per column at
once (rel err 1.813e-2 vs the 2e-2 gate; the numpy/ml_dtypes sim predicts
the HW error to ~5 decimals).
Scale bookkeeping: fp8 FFN1 operands carry x*16 and W1*64, so the bf16 W1
part is pre-scaled by 1024 on host and the gelu applies scale=1/1024; all
of W2 is pre-scaled by 128 (fp8 part stored as-is, bf16 part exact power
of 2) and the output drain applies scale=1/128.
"""

import numpy as np

B, S, D, F = 4, 4096, 2048, 8192
NUM_TOKENS = 2048
NCORES = 8
ROWS = (B * NUM_TOKENS) // NCORES     # 1024 rows per core
P = 128
KT8 = 2                               # k-subtiles over D handled in fp8
KT_B = D // P - KT8                   # 14 bf16 k-subtiles (D rows 256..2047)
FT = F // P                           # 64 f-tiles
WU = 4                                # warmup f-tiles (k-outer startup)
DT = D // P                           # 16 d-tiles
NCH = ROWS // 512                     # 2 row chunks of 512
FT8 = 6                               # f-subtiles of FFN2 handled in fp8
W2OFF = 4                             # first f-subtile held in the bf16 w2 param
FT_B = FT - W2OFF                     # 60 f-subtiles in the bf16 w2 param
                                      # (subtiles 4..5 overlap the fp8 part and
                                      # are skipped by the chain)
FSB = 4                               # w2 f-stream blocks per d (15 ft each)
SX, SW = 16.0, 64.0                   # fp8 pre-scales for x and W1
INV_S = 1.0 / (SX * SW)
S2 = 128.0                            # pre-scale for all of W2
INV_S2 = 1.0 / S2

_CACHE = {}


def _build():
    import concourse.bass as bass
    import concourse.mybir as mybir
    import concourse.tile as tile
    from concourse import bacc

    f32 = mybir.dt.float32
    bf16 = mybir.dt.bfloat16
    fp8 = mybir.dt.float8e4
    DR = mybir.MatmulPerfMode.DoubleRow

    nc = bacc.Bacc()
    xt = nc.declare_dram_parameter("xt", [KT_B, P, ROWS], bf16, isOutput=False)
    # fp8 moving operands are k-INTERLEAVED ([p, n, ks], pair adjacent in
    # memory) so the PE fetches both DoubleRow values in one 16-bit read
    xt8 = nc.declare_dram_parameter("xt8", [P, ROWS, KT8], fp8, isOutput=False)
    w1 = nc.declare_dram_parameter("w1", [FT, P, KT_B, P], bf16, isOutput=False)
    w18 = nc.declare_dram_parameter("w18", [FT, P, KT8, P], fp8, isOutput=False)
    # w2 grouped for phase B streaming: per (d, fsb) a [P, 15, P] bf16 block
    # (f-subtiles 4..63) plus a small fp8 block for f-subtiles 0..3
    w2 = nc.declare_dram_parameter("w2", [DT, FSB, P, FT_B // FSB, P], bf16,
                                   isOutput=False)
    w28 = nc.declare_dram_parameter("w28", [DT, P, FT8, P], fp8, isOutput=False)
    b1 = nc.declare_dram_parameter("b1", [P, FT], f32, isOutput=False)
    b2 = nc.declare_dram_parameter("b2", [P, DT], f32, isOutput=False)
    out = nc.declare_dram_parameter("out", [DT, P, ROWS], f32, isOutput=True)

    with tile.TileContext(nc) as tc:
        with (
            tc.tile_pool(name="resident", bufs=1) as res_pool,
            tc.tile_pool(name="wp", bufs=6) as wp,
            tc.tile_pool(name="w8p", bufs=6) as w8p,
            tc.tile_pool(name="w28p", bufs=4) as w28p,
            tc.tile_pool(name="stg", bufs=4) as stg,
            tc.tile_pool(name="ps1", bufs=6, space="PSUM") as ps1,
            tc.tile_pool(name="ps2", bufs=2, space="PSUM") as ps2,
        ):
            # residents: xt bf16 (28KB/part), xt8 fp8 (2KB/part), H (128KB/part)
            xt_all = res_pool.tile([P, KT_B, ROWS], bf16, name="xt_all")
            xt8_sb = res_pool.tile([P, ROWS, KT8], fp8, name="xt8sb")
            ht_sb = [res_pool.tile([P, ROWS], bf16, name=f"ht{ft}") for ft in range(FT)]
            # fp8 copies of ht f-subtiles 0..3, paired for DoubleRow
            ht8_sb = [res_pool.tile([P, ROWS, 2], fp8, name=f"ht8p{j}")
                      for j in range(FT8 // 2)]
            w1_warm = [wp.tile([P, KT_B * P], bf16, name=f"w1t_{ft}", tag="wp")
                       for ft in range(WU)]
            w18_warm = [w8p.tile([P, KT8, P], fp8, name=f"w18t_{ft}", tag="w8p")
                        for ft in range(WU)]
            b1_sb = res_pool.tile([P, FT], f32, name="b1sb")
            b2_sb = res_pool.tile([P, DT], f32, name="b2sb")

            # Startup DMAs: trigger instructions serialize at ~600ns each on
            # the Sync engine, so order them first-needed first (each chain
            # starts with its fp8 DoubleRow matmul: w18 + xt8 go first).
            nc.sync.dma_start(out=w18_warm[0][:], in_=w18[0])
            nc.sync.dma_start(out=w18_warm[1][:], in_=w18[1])
            nc.sync.dma_start(out=xt8_sb[:], in_=xt8[:])
            nc.sync.dma_start(out=w1_warm[0][:], in_=w1[0].rearrange("p k c -> p (k c)"))
            nc.sync.dma_start(out=xt_all[:, 0, :], in_=xt[0])
            nc.sync.dma_start(out=w1_warm[1][:], in_=w1[1].rearrange("p k c -> p (k c)"))
            nc.sync.dma_start(out=xt_all[:, 1, :], in_=xt[1])
            nc.sync.dma_start(out=b1_sb[:], in_=b1[:])
            for k in range(2, KT_B):
                nc.sync.dma_start(out=xt_all[:, k, :], in_=xt[k])
            nc.sync.dma_start(out=w18_warm[2][:], in_=w18[2])
            nc.sync.dma_start(out=w18_warm[3][:], in_=w18[3])
            nc.sync.dma_start(out=w1_warm[2][:], in_=w1[2].rearrange("p k c -> p (k c)"))
            nc.sync.dma_start(out=w1_warm[3][:], in_=w1[3].rearrange("p k c -> p (k c)"))
            nc.sync.dma_start(out=b2_sb[:], in_=b2[:])

            def ffn1_chain(psum, w18_t, w1_t, n, for_k=None):
                """Issue one (ft, n) FFN1 chain; for_k limits to one bf16 k."""
                if for_k is None or for_k == -1:
                    nc.tensor.matmul(
                        psum[:], w18_t[:],
                        xt8_sb[:, n * 512:(n + 1) * 512, :].rearrange("p n k -> p k n"),
                        start=True, stop=False, perf_mode=DR,
                    )
                ks = range(KT_B) if for_k is None else (
                    [] if for_k == -1 else [for_k])
                for k in ks:
                    nc.tensor.matmul(
                        psum[:],
                        w1_t[:, k * P:(k + 1) * P],
                        xt_all[:, k, n * 512:(n + 1) * 512],
                        start=False, stop=(k == KT_B - 1),
                    )

            # ---------- phase A: H = gelu((x @ W1)/1024 + b1) ----------
            # warmup block: k-outer over 4 concurrent psum chains so matmuls
            # start as soon as xt_all[:, k] lands instead of waiting for all XT.
            for half in range(2):
                chains = [(half * 2 + i, n) for i in range(2) for n in range(NCH)]
                psums = {
                    c: ps1.tile([P, 512], f32, name=f"ps1w_{c[0]}_{c[1]}", tag="ps1")
                    for c in chains
                }
                for fs, n in chains:
                    ffn1_chain(psums[(fs, n)], w18_warm[fs], None, n, for_k=-1)
                for k in range(KT_B):
                    for fs, n in chains:
                        ffn1_chain(psums[(fs, n)], None, w1_warm[fs], n, for_k=k)
                for fs, n in chains:
                    # phase B consumes f-subtiles < FT8 in fp8 (DoubleRow) and
                    # the rest in bf16, so produce exactly the copy it reads
                    if fs < FT8:
                        nc.scalar.activation(
                            ht8_sb[fs // 2][:, n * 512:(n + 1) * 512, fs % 2],
                            psums[(fs, n)][:],
                            mybir.ActivationFunctionType.Gelu_apprx_tanh,
                            bias=b1_sb[:, fs:fs + 1], scale=INV_S,
                        )
                    else:
                        nc.scalar.activation(
                            ht_sb[fs][:, n * 512:(n + 1) * 512], psums[(fs, n)][:],
                            mybir.ActivationFunctionType.Gelu_apprx_tanh,
                            bias=b1_sb[:, fs:fs + 1], scale=INV_S,
                        )

            for ft in range(WU, FT):
                w18_t = w8p.tile([P, KT8, P], fp8, name=f"w18t_{ft}", tag="w8p")
                nc.sync.dma_start(out=w18_t[:], in_=w18[ft])
                w1_t = wp.tile([P, KT_B * P], bf16, name=f"w1t_{ft}", tag="wp")
                nc.sync.dma_start(out=w1_t[:], in_=w1[ft].rearrange("p k c -> p (k c)"))
                for n in range(NCH):
                    psum = ps1.tile([P, 512], f32, name=f"ps1_{ft}_{n}", tag="ps1")
                    ffn1_chain(psum, w18_t, w1_t, n)
                    if ft < FT8:
                        ht_out = ht8_sb[ft // 2][:, n * 512:(n + 1) * 512, ft % 2]
                    else:
                        ht_out = ht_sb[ft][:, n * 512:(n + 1) * 512]
                    nc.scalar.activation(
                        ht_out, psum[:],
                        mybir.ActivationFunctionType.Gelu_apprx_tanh,
                        bias=b1_sb[:, ft:ft + 1], scale=INV_S,
                    )

            # ---------- phase B: out[d] = (H @ W2*128)/128 + b2 ----------
            # per (d, n): 2 fp8 DoubleRow matmuls (f-subtiles 0..3) + 60 bf16
            for d in range(DT):
                w28_t = w28p.tile([P, FT8, P], fp8, name=f"w28t_{d}", tag="w28p")
                nc.sync.dma_start(out=w28_t[:], in_=w28[d])
                w2_sb = [wp.tile([P, (FT_B // FSB) * P], bf16, name=f"w2t_{d}_{fsb}",
                                 tag="wp") for fsb in range(FSB)]
                for fsb in range(FSB):
                    nc.sync.dma_start(
                        out=w2_sb[fsb][:],
                        in_=w2[d, fsb].rearrange("p k c -> p (k c)"))
                for n in range(NCH):
                    psum2 = ps2.tile([P, 512], f32, name=f"ps2_{d}_{n}", tag="ps2")
                    for j in range(FT8 // 2):
                        nc.tensor.matmul(
                            psum2[:],
                            w28_t[:, 2 * j:2 * j + 2, :],
                            ht8_sb[j][:, n * 512:(n + 1) * 512, :].rearrange("p n k -> p k n"),
                            start=(j == 0), stop=False, perf_mode=DR,
                        )
                    for fs in range(FT8, FT):
                        fsb, fi = divmod(fs - W2OFF, FT_B // FSB)
                        nc.tensor.matmul(
                            psum2[:],
                            w2_sb[fsb][:, fi * P:(fi + 1) * P],
                            ht_sb[fs][:, n * 512:(n + 1) * 512],
                            start=False, stop=(fs == FT - 1),
                        )
                    o_sb = stg.tile([P, 512], f32, name=f"o_{d}_{n}", tag="stg")
                    nc.scalar.activation(
                        o_sb[:], psum2[:],
                        mybir.ActivationFunctionType.Identity,
                        bias=b2_sb[:, d:d + 1], scale=INV_S2,
                    )
                    nc.sync.dma_start(out=out[d, :, n * 512:(n + 1) * 512], in_=o_sb[:])

    nc.compile()
    return nc


def _get_nc():
    if "nc" not in _CACHE:
        _CACHE["nc"] = _build()
    return _CACHE["nc"]


def _prep_in_maps(x, W1, b1, W2, b2):
    """Host-side shard + layout prep. Returns in_maps for the 8 cores."""
    import ml_dtypes

    BF16 = ml_dtypes.bfloat16
    FP8 = ml_dtypes.float8_e4m3
    x = np.asarray(x, dtype=np.float32)
    W1 = np.asarray(W1, dtype=np.float32)
    W2 = np.asarray(W2, dtype=np.float32)
    b1 = np.asarray(b1, dtype=np.float32)
    b2 = np.asarray(b2, dtype=np.float32)

    KCUT = KT8 * P                                               # 256
    xs = x[:, :NUM_TOKENS, :].reshape(B * NUM_TOKENS, D)         # [8192, 2048]
    # bf16 W1 part pre-scaled by SX*SW (power of 2: exact in bf16)
    w1h = np.ascontiguousarray(
        (W1[KCUT:] * (SX * SW)).reshape(KT_B, P, FT, P)
        .transpose(2, 1, 0, 3)).astype(BF16)                     # [ft, p, k, c]
    w18h = np.ascontiguousarray(
        (W1[:KCUT] * SW).reshape(KT8, P, FT, P)
        .transpose(2, 1, 0, 3)).astype(FP8)                      # [ft, p, k8, c]
    # all of W2 carries the S2 scale (exact power of 2 for the bf16 part);
    # the bf16 param keeps f-subtiles 4..63 (4..5 unused), fp8 covers 0..5
    w2h = np.ascontiguousarray(
        (W2[W2OFF * P:] * S2).reshape(FSB, FT_B // FSB, P, DT, P)
        .transpose(3, 0, 2, 1, 4)).astype(BF16)                  # [d, blk, p, fi, c]
    w28h = np.ascontiguousarray(
        (W2[:FT8 * P] * S2).reshape(FT8, P, DT, P)
        .transpose(2, 1, 0, 3)).astype(FP8)                      # [d, p, ks, c]
    b1h = np.ascontiguousarray(b1.reshape(FT, P).T)              # [p, ft]
    b2h = np.ascontiguousarray(b2.reshape(DT, P).T)              # [p, d]

    in_maps = []
    for c in range(NCORES):
        xc = xs[c * ROWS:(c + 1) * ROWS]                         # [1024, 2048]
        xth = np.ascontiguousarray(
            xc[:, KCUT:].T.reshape(KT_B, P, ROWS)).astype(BF16)
        # [p, n, k8]: the two DoubleRow values adjacent in memory per column
        xt8h = np.ascontiguousarray(
            (xc[:, :KCUT] * SX).reshape(ROWS, KT8, P)
            .transpose(2, 0, 1)).astype(FP8)
        in_maps.append({"xt": xth, "xt8": xt8h, "w1": w1h, "w18": w18h,
                        "w2": w2h, "w28": w28h, "b1": b1h, "b2": b2h})
    return in_maps


def _gather_out(results):
    out = np.empty((B * NUM_TOKENS, D), dtype=np.float32)
    for c in range(NCORES):
        oc = results[c]["out"]                                   # [d, p, n]
        out[c * ROWS:(c + 1) * ROWS] = oc.reshape(D, ROWS).T
    return out.reshape(B, NUM_TOKENS, D)


def kernel(x, Wp, bp, W1, b1, W2, b2, **_unused):
    from concourse.bass_utils import run_bass_kernel_spmd

    in_maps = _prep_in_maps(x, W1, b1, W2, b2)
    nc = _get_nc()
    res = run_bass_kernel_spmd(nc, in_maps, list(range(NCORES)))
    return _gather_out(res.results)
